# revision 1
# baseline (speedup 1.0000x reference)
"""Trainium2 Bass kernel for LocalWindowAttention.

Reference semantics (per batch b):
    pad seq 4000 -> 4096, split into 32 windows of 128 tokens.
    qkv = x @ w_qkv.T + b_qkv ; per-window per-head softmax(q k^T / sqrt(64)) @ v
    out = o @ w_out.T + b_out ; drop padded tail.

Sharding: data-parallel over batch. Core b computes batch b fully.

Per-core layout strategy (everything chosen so matmul contraction = partition dim):
  - x is staged feature-major  xT[e, t]  (e on partitions, 8 chunks of 128).
  - Q computed feature-major (f on partitions); K likewise but stored per-head
    zero-padded to the full 128 partitions (kz) so every score matmul reads
    inputs at base partition 0:
        S^T[tk, tq] = sum_d K[d, tk] Q[d, tq]   (lhsT=kz_h, rhs=Q pair, K=128)
  - V computed token-major (t on partitions) so AV works with V as stationary:
        O_u[d, tq] = sum_tk V[tk, d] E[tk, tq]
  - softmax denominators: 16 accumulating one-hot matmuls (stream-bound: the
    one-hot stationary is only 16 columns wide) give D16[h, tq]; reciprocal
    on DVE, broadcast back to O shape via a (16 x 128) selector matmul, then
    one DVE multiply normalizes O.  (1/sqrt(64) is folded into w_q on the
    host; exp is computed without max-subtraction which is exact for softmax
    and safe here: |scores| <= ~3.)
  - out projection consumes O feature-major chunks directly.
  - tail trim: only 4000 of 4096 tokens are real. The last chunk's Q matmuls
    stream 416 columns instead of 512 and the last window's attention
    matmuls stream 32 query columns instead of 128.
All matmuls use bf16/fp16 operands (1 cycle/row on TRN2; fp32 is 4x slower).
Accumulation is always fp32 in PSUM.
"""

import sys
import numpy as np

for _p in ("/opt/trn_rl_repo", "/root/.axon_site/_ro/trn_rl_repo"):
    if _p not in sys.path:
        sys.path.append(_p)

import ml_dtypes

P = 128          # partitions
E = 1024         # embed dim
H = 16           # heads
D = 64           # head dim
W = 128          # window
B = 8            # batch
S = 4000         # seq len
SP = 4096        # padded seq len
NW = SP // W     # 32 windows
CW = 4           # windows per chunk
CT = CW * W      # 512 tokens per chunk
EC = 8           # e-chunks of 128

BF16 = ml_dtypes.bfloat16
F16 = np.float16

_cache = {}


def build_nc(n_chunks, s_out, has_bqk, has_bout):
    """Build + compile the single-core Bass program (same program for all cores)."""
    from concourse import bacc, tile, mybir

    dt = mybir.dt
    AF = mybir.ActivationFunctionType

    nc = bacc.Bacc(None, target_bir_lowering=False, debug=False)

    xt_d = nc.dram_tensor("xt", [n_chunks, P, EC, CT], dt.bfloat16, kind="ExternalInput")
    wqkv_d = nc.dram_tensor("wqkv", [P, EC, 3 * E], dt.bfloat16, kind="ExternalInput")
    wout_d = nc.dram_tensor("wout", [P, EC, E], dt.bfloat16, kind="ExternalInput")
    oh_d = nc.dram_tensor("onehot", [P, H, H], dt.bfloat16, kind="ExternalInput")
    sel_d = nc.dram_tensor("sel", [H, EC, P], dt.float16, kind="ExternalInput")
    out_d = nc.dram_tensor("out", [s_out, E], dt.float32, kind="ExternalOutput")
    if has_bqk:
        bqk_d = nc.dram_tensor("bqk", [P, 2, EC], dt.float32, kind="ExternalInput")
    if has_bout:
        cb_d = nc.dram_tensor("cb", [P, 2, 512], dt.float32, kind="ExternalInput")

    with tile.TileContext(nc) as tc:
        with (
            tc.tile_pool(name="const", bufs=1) as constp,
            tc.tile_pool(name="xp", bufs=2) as xp,
            tc.tile_pool(name="qkp", bufs=2) as qkp,
            tc.tile_pool(name="kzp", bufs=1) as kzp,
            tc.tile_pool(name="ktp", bufs=3) as ktp,
            tc.tile_pool(name="vp", bufs=2) as vp,
            tc.tile_pool(name="ep", bufs=2) as ep,
            tc.tile_pool(name="op", bufs=2) as opool,
            tc.tile_pool(name="rp", bufs=2) as rp,
            tc.tile_pool(name="fpl", bufs=3) as fpl,
            tc.tile_pool(name="psA", bufs=4, space="PSUM") as psA,
        ):
            # startup-critical DMAs first. wqQ / chunk-0 x live in per-ec
            # TILES (tile-granular dependency tracking) so chunk 0's first
            # ec-outer Q matmul unblocks after two small transfers instead
            # of the full 8.4MB of weights.
            wqQ = [constp.tile([P, E], dt.bfloat16, name=f"wqQ{ec}")
                   for ec in range(EC)]
            xt0 = [constp.tile([P, CT], dt.bfloat16, name=f"xt0_{ec}")
                   for ec in range(EC)]
            # xt0[0] first (its DMA queue spins up first) and wqQ[0]'s
            # first f-tile in its own small DMA: the opening matmul's deps
            # are then one 128KB and one 32KB transfer.
            nc.sync.dma_start(xt0[0][:], xt_d[0][:, 0, :])
            nc.sync.dma_start(wqQ[0][:, 0:P], wqkv_d[:, 0, 0:P])
            nc.sync.dma_start(wqQ[0][:, P:E], wqkv_d[:, 0, P:E])
            for ec in range(1, EC):
                nc.sync.dma_start(wqQ[ec][:], wqkv_d[:, ec, 0:E])
                nc.sync.dma_start(xt0[ec][:], xt_d[0][:, ec, :])
            wq = constp.tile([P, EC, 2 * E], dt.bfloat16)  # K and V blocks
            for ec in range(EC):
                nc.sync.dma_start(wq[:, ec, 0:E], wqkv_d[:, ec, E:2 * E])
            for ec in range(EC):
                nc.sync.dma_start(wq[:, ec, E:2 * E], wqkv_d[:, ec, 2 * E:3 * E])
            oh = constp.tile([P, H, H], dt.bfloat16)
            nc.sync.dma_start(oh[:], oh_d[:])
            sel = constp.tile([H, EC, P], dt.float16)
            nc.sync.dma_start(sel[:], sel_d[:])
            wo = constp.tile([P, EC, E], dt.bfloat16)
            for ec in range(EC):
                nc.sync.dma_start(wo[:, ec, :], wout_d[:, ec, :])
            if has_bqk:
                bqk = constp.tile([P, 2, EC], dt.float32)
                nc.sync.dma_start(bqk[:], bqk_d[:])
            if has_bout:
                cb = constp.tile([P, 2, 512], dt.float32)
                nc.sync.dma_start(cb[:], cb_d[:])

            # kz zero halves never change: clear the two persistent tiles once.
            kz_tiles = []
            for i in range(2):
                kzt = kzp.tile([P, H, CT], dt.bfloat16, tag=f"kz{i}", name=f"kz{i}")
                nc.gpsimd.memset(kzt[:], 0.0)
                kz_tiles.append(kzt)

            def stage_a1(wi, kz_sb, q_sb, tq):
                """scores -> exp (quartered ACTs so the D chain can start early)."""
                e_sb = ep.tile([P, H, W], dt.bfloat16, tag="e")
                for half in range(2):
                    ps_s = psA.tile([P, 8, W], dt.float32, tag="ps")
                    for j in range(8):
                        h = half * 8 + j
                        # kz's invalid half is zero, so contracting all 128
                        # rows against the Q f-tile pair selects head h.
                        nc.tensor.matmul(
                            ps_s[:, j, :tq],
                            kz_sb[:, h, wi * W:(wi + 1) * W],
                            q_sb[:, h // 2, wi * W:wi * W + tq],
                            start=True,
                            stop=True,
                        )
                    for qq in range(2):
                        nc.scalar.activation(
                            e_sb[:, half * 8 + qq * 4:half * 8 + qq * 4 + 4, :tq],
                            ps_s[:, qq * 4:qq * 4 + 4, :tq], AF.Exp,
                        )
                return e_sb

            def stage_d16(e_sb, tq):
                """denominators D16[h, tq] via one-hot matmuls (the one-hot
                stationary is only 16 columns, so these are stream-bound),
                then recip -> f16 cast for the broadcast matmul."""
                ps_d = psA.tile([H, W], dt.float32, tag="ps")
                for h in range(H):
                    nc.tensor.matmul(
                        ps_d[:, :tq], oh[:, h, :], e_sb[:, h, :tq],
                        start=(h == 0), stop=(h == H - 1),
                    )
                rd32 = rp.tile([H, W], dt.float32, tag="rd32")
                nc.vector.reciprocal_approx_fast(rd32[:, :tq], ps_d[:, :tq])
                rd16 = rp.tile([H, W], dt.float16, tag="rd16")
                nc.vector.tensor_copy(rd16[:, :tq], rd32[:, :tq])
                return rd16

            def stage_a2(wi, e_sb, v_sb, rd16, tq):
                """bcast recip -> AV (r evicted during AV) -> normalized O."""
                # broadcast recip to O shape: R_O[cc*128+p, tq] = rd[2cc+p//64, tq]
                ps_r = psA.tile([P, EC, W], dt.float32, tag="ps")
                for cc in range(EC):
                    nc.tensor.matmul(
                        ps_r[:, cc, :tq], sel[:, cc, :], rd16[:, :tq],
                        start=True, stop=True,
                    )
                r_sb = rp.tile([P, EC, W], dt.float16, tag="ro")
                nc.vector.tensor_copy(r_sb[:, :, :tq], ps_r[:, :, :tq])
                # AV halves write two SEPARATE psum tiles so o_h0's multiply
                # (DVE) can run while AV's second half still streams on PE --
                # the next window's out-projection then finds o_h0 ready.
                o_halves = []
                for hh in range(2):
                    ps_o = psA.tile([P, 4, W], dt.float32, tag="ps",
                                    name=f"ps_o{hh}")
                    for h in range(hh * 8, hh * 8 + 8):
                        cc = h // 2 - hh * 4
                        po = (h % 2) * D
                        nc.tensor.matmul(
                            ps_o[po:po + D, cc, :tq],
                            v_sb[:, wi, h // 8, (h % 8) * D:(h % 8) * D + D],
                            e_sb[:, h, :tq],
                            start=True,
                            stop=True,
                        )
                    sl = slice(hh * 4, hh * 4 + 4)
                    o_h = opool.tile([P, 4, W], dt.bfloat16, tag=f"o{hh}",
                                     name=f"o{hh}")
                    nc.vector.tensor_mul(o_h[:, :, :tq], ps_o[:, :, :tq],
                                         r_sb[:, sl, :tq])
                    o_halves.append(o_h)
                return o_halves

            def bmm_one(ps_f, o_halves, fh, cc):
                nc.tensor.matmul(
                    ps_f[:, fh, :],
                    o_halves[cc // 4][:, cc % 4, :],
                    wo[:, cc, fh * 512:(fh + 1) * 512],
                    start=(cc == 0),
                    stop=(cc == EC - 1),
                )

            def stage_b_out_half(ps_f, row0, rows, fh):
                """evict + DMA one 512-feature half of the out projection."""
                f_sb = fpl.tile([P, 512], dt.float32, tag=f"f{fh}", name=f"f{fh}")
                if has_bout:
                    nc.vector.tensor_add(f_sb[:], ps_f[:, fh, :], cb[:, fh, :])
                else:
                    # on ScalarE: keeps DVE free for the recip/normalize chain
                    nc.scalar.activation(f_sb[:], ps_f[:, fh, :], AF.Copy)
                nc.sync.dma_start(
                    out_d[row0:row0 + rows, fh * 512:(fh + 1) * 512],
                    f_sb[:rows],
                )

            pend = None
            for c in range(n_chunks):
                tcv = min(s_out - c * CT, CT)  # valid tokens in this chunk
                if c == 0:
                    xt = None
                else:
                    xt = xp.tile([P, EC, CT], dt.bfloat16, tag="xt")
                    nc.sync.dma_start(xt[:], xt_d[c])

                def xt_ec(ec, sl=slice(None)):
                    return xt0[ec][:, sl] if c == 0 else xt[:, ec, sl]

                q_sb = qkp.tile([P, EC, CT], dt.bfloat16, tag="q")
                kz_sb = kz_tiles[c % 2]
                v_sb = vp.tile([P, CW, 2, 512], dt.bfloat16, tag="v")

                # ---- Q (feature-major): psum[f_tile, t] ----
                if c == 0:
                    # ec-outer so the first matmul only needs the first two
                    # small DMAs; uses all 4 psum slots as live accumulators.
                    ps_qs = [psA.tile([P, 2, 512], dt.float32, tag="ps",
                                      name=f"ps_q{i}")
                             for i in range(4)]
                    for ec in range(EC):
                        for fg in range(4):
                            for half in range(2):
                                ft = fg * 2 + half
                                nc.tensor.matmul(
                                    ps_qs[fg][:, half, :tcv],
                                    wqQ[ec][:, ft * P:ft * P + P],
                                    xt_ec(ec, slice(0, tcv)),
                                    start=(ec == 0),
                                    stop=(ec == EC - 1),
                                )
                    for fg in range(4):
                        ps = ps_qs[fg]
                        if has_bqk:
                            for half in range(2):
                                ft = fg * 2 + half
                                nc.scalar.activation(
                                    q_sb[:, ft, :tcv], ps[:, half, :tcv],
                                    AF.Identity, bias=bqk[:, 0, ft:ft + 1],
                                )
                        else:
                            nc.scalar.activation(
                                q_sb[:, fg * 2:fg * 2 + 2, :tcv],
                                ps[:, :, :tcv], AF.Copy,
                            )
                else:
                    for fg in range(4):
                        ps = psA.tile([P, 2, 512], dt.float32, tag="ps")
                        for half in range(2):
                            ft = fg * 2 + half
                            for ec in range(EC):
                                nc.tensor.matmul(
                                    ps[:, half, :tcv],
                                    wqQ[ec][:, ft * P:ft * P + P],
                                    xt_ec(ec, slice(0, tcv)),
                                    start=(ec == 0),
                                    stop=(ec == EC - 1),
                                )
                        if has_bqk:
                            for half in range(2):
                                ft = fg * 2 + half
                                nc.scalar.activation(
                                    q_sb[:, ft, :tcv], ps[:, half, :tcv],
                                    AF.Identity, bias=bqk[:, 0, ft:ft + 1],
                                )
                        else:
                            nc.scalar.activation(
                                q_sb[:, fg * 2:fg * 2 + 2, :tcv],
                                ps[:, :, :tcv], AF.Copy,
                            )

                # ---- K (feature-major, full 512 cols: padded tokens give the
                # zero keys the reference semantics require) ----
                for fg in range(4):
                    ps = psA.tile([P, 2, 512], dt.float32, tag="ps")
                    for half in range(2):
                        ft = fg * 2 + half
                        off = ft * P
                        for ec in range(EC):
                            nc.tensor.matmul(
                                ps[:, half, :],
                                wq[:, ec, off:off + P],
                                xt_ec(ec),
                                start=(ec == 0),
                                stop=(ec == EC - 1),
                            )
                    ktmp = ktp.tile([P, 2, 512], dt.bfloat16, tag="kt")
                    if has_bqk:
                        for half in range(2):
                            ft = fg * 2 + half
                            nc.scalar.activation(
                                ktmp[:, half, :], ps[:, half, :],
                                AF.Identity, bias=bqk[:, 1, ft:ft + 1],
                            )
                    else:
                        nc.scalar.activation(ktmp[:], ps[:], AF.Copy)
                    for half in range(2):
                        ft = fg * 2 + half
                        for hh in range(2):
                            pr = slice(hh * 64, hh * 64 + 64)
                            nc.sync.dma_start(
                                kz_sb[pr, 2 * ft + hh, :],
                                ktmp[pr, half, :],
                            )

                # ---- V (token-major): psum[t, f] per window; evicted on the
                # otherwise-idle Pool engine so DVE stays free for the
                # recip/normalize chain. ----
                for wi in range(CW):
                    ps = psA.tile([P, 2, 512], dt.float32, tag="ps")
                    for fh in range(2):
                        off = E + fh * 512
                        for ec in range(EC):
                            nc.tensor.matmul(
                                ps[:, fh, :],
                                xt_ec(ec, slice(wi * W, (wi + 1) * W)),
                                wq[:, ec, off:off + 512],
                                start=(ec == 0),
                                stop=(ec == EC - 1),
                            )
                    nc.vector.tensor_copy(v_sb[:, wi], ps[:])

                # ---- attention (A) + out-projection (B), software-pipelined:
                # B(w-1) is emitted inside A(w) so the PE has big streams to
                # hide the D-chain ldweights and the evict->normalize latency.
                for wi in range(CW):
                    g = c * CW + wi
                    row0 = g * W
                    rows = min(s_out - row0, W)
                    if rows <= 0:
                        continue
                    tq = rows
                    e_sb = stage_a1(wi, kz_sb, q_sb, tq)
                    if pend is not None:
                        ps_f = psA.tile([P, 2, 512], dt.float32, tag="ps")
                        for cc in range(EC):
                            bmm_one(ps_f, pend[0], 0, cc)
                    rd16 = stage_d16(e_sb, tq)
                    if pend is not None:
                        stage_b_out_half(ps_f, pend[1], pend[2], 0)
                        for cc in range(EC):
                            bmm_one(ps_f, pend[0], 1, cc)
                        stage_b_out_half(ps_f, pend[1], pend[2], 1)
                    o_halves = stage_a2(wi, e_sb, v_sb, rd16, tq)
                    pend = (o_halves, row0, rows)

            if pend is not None:
                ps_f = psA.tile([P, 2, 512], dt.float32, tag="ps")
                for cc in range(EC):
                    bmm_one(ps_f, pend[0], 0, cc)
                stage_b_out_half(ps_f, pend[1], pend[2], 0)
                for cc in range(EC):
                    bmm_one(ps_f, pend[0], 1, cc)
                stage_b_out_half(ps_f, pend[1], pend[2], 1)

    nc.compile()
    return nc


def prep_inputs(x, w_qkv, b_qkv, w_out, b_out, n_chunks, s_out):
    """Host-side staging: pad, transpose, cast, fold scale into w_q."""
    sp = n_chunks * CT
    nb = x.shape[0]

    wqkvT = np.ascontiguousarray(w_qkv.T).astype(np.float32).copy()
    wqkvT[:, :E] *= 1.0 / np.sqrt(D)
    wqkv_sb = np.ascontiguousarray(
        wqkvT.reshape(EC, P, 3 * E).transpose(1, 0, 2)
    ).astype(BF16)

    woutT = np.ascontiguousarray(w_out.T)
    wout_sb = np.ascontiguousarray(
        woutT.reshape(EC, P, E).transpose(1, 0, 2)
    ).astype(BF16)

    oh = np.zeros((P, H, H), dtype=BF16)
    for h in range(H):
        oh[:, h, h] = 1
    selm = np.zeros((H, EC, P), dtype=F16)
    for cc in range(EC):
        for m in range(P):
            selm[2 * cc + m // D, cc, m] = 1

    base = {"wqkv": wqkv_sb, "wout": wout_sb, "onehot": oh, "sel": selm}

    has_bqk = bool(np.any(b_qkv[:2 * E]))
    has_bout = bool(np.any(b_out)) or bool(np.any(b_qkv[2 * E:]))
    if has_bqk:
        bqk = np.stack(
            [b_qkv[:E].reshape(EC, P).T / np.sqrt(D),
             b_qkv[E:2 * E].reshape(EC, P).T], axis=1
        ).astype(np.float32)  # (P, 2, EC)
        base["bqk"] = np.ascontiguousarray(bqk)
    if has_bout:
        cbv = (b_out + b_qkv[2 * E:] @ w_out.T).astype(np.float32)  # (E,)
        base["cb"] = np.ascontiguousarray(
            np.broadcast_to(cbv.reshape(1, 2, 512), (P, 2, 512))
        ).copy()

    in_maps = []
    for b in range(nb):
        xp_ = np.zeros((sp, E), dtype=np.float32)
        xp_[:min(s_out, x.shape[1])] = x[b][:s_out]
        xT = np.ascontiguousarray(xp_.T)  # (E, sp)
        xt_sb = np.ascontiguousarray(
            xT.reshape(EC, P, n_chunks, CT).transpose(2, 1, 0, 3)
        ).astype(BF16)  # (n_chunks, P, EC, CT)
        m = dict(base)
        m["xt"] = xt_sb
        in_maps.append(m)
    return in_maps, has_bqk, has_bout


def run(x, w_qkv, b_qkv, w_out, b_out, n_chunks=NW // CW, s_out=S, trace=False):
    from concourse import bass_utils

    in_maps, has_bqk, has_bout = prep_inputs(
        x, w_qkv, b_qkv, w_out, b_out, n_chunks, s_out
    )
    key = (n_chunks, s_out, has_bqk, has_bout)
    if key not in _cache:
        _cache[key] = build_nc(*key)
    nc = _cache[key]

    res = bass_utils.run_bass_kernel_spmd(
        nc, in_maps, core_ids=list(range(len(in_maps))), trace=trace,
    )
    out = np.stack([r["out"] for r in res.results], axis=0)
    return out, res


def kernel(x, w_qkv, b_qkv, w_out, b_out):
    x = np.asarray(x, dtype=np.float32)
    w_qkv = np.asarray(w_qkv, dtype=np.float32)
    b_qkv = np.asarray(b_qkv, dtype=np.float32)
    w_out = np.asarray(w_out, dtype=np.float32)
    b_out = np.asarray(b_out, dtype=np.float32)
    out, _ = run(x, w_qkv, b_qkv, w_out, b_out)
    return out



# revision 10
# speedup vs baseline: 1.0050x; 1.0050x over previous
"""Trainium2 Bass kernel for LocalWindowAttention.

Reference semantics (per batch b):
    pad seq 4000 -> 4096, split into 32 windows of 128 tokens.
    qkv = x @ w_qkv.T + b_qkv ; per-window per-head softmax(q k^T / sqrt(64)) @ v
    out = o @ w_out.T + b_out ; drop padded tail.

Sharding: data-parallel over batch. Core b computes batch b fully.

Per-core layout strategy (everything chosen so matmul contraction = partition dim):
  - x is staged feature-major  xT[e, t]  (e on partitions, 8 chunks of 128).
  - Q computed feature-major (f on partitions); K likewise, kept in its
    eviction layout (f on partitions = head-pairs of d). With PACK_SCORES the
    score matmuls are K=64 contractions at base partition 0/64 so head pairs
    run concurrently in the two PE row-groups (row tiling); otherwise K is
    also staged zero-padded per head (kz) for K=128 scores.
  - V computed token-major (t on partitions) so AV works with V as stationary:
        O_u[d, tq] = sum_tk V[tk, d] E[tk, tq]      (col-tiled pairs, pos (0, 0/64))
  - softmax denominators via one-hot matmuls; with PACK_DB two 8-matmul
    accumulation groups col-tiled at PE columns 0/64 (separate PSUM banks)
    run 2-way concurrent, and the recip broadcast back to O shape uses K=8
    selector matmuls row-tiled at PE rows 0/64.  (1/sqrt(64) is folded into
    w_q on the host; exp is computed without max-subtraction which is exact
    for softmax and safe here: |scores| <= ~3.)
  - out projection consumes O feature-major chunks directly.
  - tail trim: only 4000 of 4096 tokens are real. The last chunk's Q and K
    matmuls stream 416 columns instead of 512 (K's padded key columns are
    memset to zero so padded keys still contribute exp(0)=1 with v=0, as the
    reference requires) and the last window's attention matmuls stream 32
    query columns instead of 128.
  - ~48 dummy matmuls on a zeroed tile run during the initial weight DMA so
    the PE HAM clock-gate is already un-throttled (2.4 GHz) when real matmuls
    start.
All matmuls use bf16/fp16 operands (1 cycle/row on TRN2; fp32 is 4x slower).
Accumulation is always fp32 in PSUM.
"""

import sys
import numpy as np

for _p in ("/opt/trn_rl_repo", "/root/.axon_site/_ro/trn_rl_repo"):
    if _p not in sys.path:
        sys.path.append(_p)

import ml_dtypes

P = 128          # partitions
E = 1024         # embed dim
H = 16           # heads
D = 64           # head dim
W = 128          # window
B = 8            # batch
S = 4000         # seq len
SP = 4096        # padded seq len
NW = SP // W     # 32 windows
CW = 4           # windows per chunk
CT = CW * W      # 512 tokens per chunk
EC = 8           # e-chunks of 128

BF16 = ml_dtypes.bfloat16
F16 = np.float16

# hardware-bisect flags: each packs small attention matmuls concurrently via
# tile_position; when off, the original serial constructs are used.
PACK_SCORES = True
PACK_DB = False     # den: 2-way col-tiled accumulation groups (BROKEN on HW)
PACK_BCAST = True   # bcast: 2-way row-tiled K=16 (needs PACK_DB); else K=128
WARMUP = True

# slot permutation within a half (PACK_SCORES): concurrent score pairs
# (pos 2i, 2i+1) write psum slots (i, i+4) so the two concurrent drains hit
# different PSUM banks. e-slot of head h = (h//8)*8 + SLOT[h%8].
SLOT = [0, 4, 1, 5, 2, 6, 3, 7] if PACK_SCORES else list(range(8))
POS = [SLOT.index(s) for s in range(8)]  # slot -> pos

_cache = {}


def build_nc(n_chunks, s_out, has_bqk, has_bout):
    """Build + compile the single-core Bass program (same program for all cores)."""
    from concourse import bacc, tile, mybir

    dt = mybir.dt
    AF = mybir.ActivationFunctionType

    nc = bacc.Bacc(None, target_bir_lowering=False, debug=False)

    xt_d = nc.dram_tensor("xt", [n_chunks, P, EC, CT], dt.bfloat16, kind="ExternalInput")
    wqkv_d = nc.dram_tensor("wqkv", [P, EC, 3 * E], dt.bfloat16, kind="ExternalInput")
    wout_d = nc.dram_tensor("wout", [P, EC, E], dt.bfloat16, kind="ExternalInput")
    oh_d = nc.dram_tensor("onehot", [P, H, H], dt.bfloat16, kind="ExternalInput")
    out_d = nc.dram_tensor("out", [s_out, E], dt.float32, kind="ExternalOutput")
    if PACK_DB and PACK_BCAST:
        sel2_d = nc.dram_tensor("sel2", [P, 4, P], dt.float16, kind="ExternalInput")
    elif PACK_DB:
        sel2_d = nc.dram_tensor("sel2", [P, EC, P], dt.float16, kind="ExternalInput")
    else:
        sel_d = nc.dram_tensor("sel", [H, EC, P], dt.float16, kind="ExternalInput")
    if has_bqk:
        bqk_d = nc.dram_tensor("bqk", [P, 2, EC], dt.float32, kind="ExternalInput")
    if has_bout:
        cb_d = nc.dram_tensor("cb", [P, 2, 512], dt.float32, kind="ExternalInput")

    with tile.TileContext(nc) as tc:
        with (
            tc.tile_pool(name="const", bufs=1) as constp,
            tc.tile_pool(name="xp", bufs=2) as xp,
            tc.tile_pool(name="qkp", bufs=2) as qkp,
            tc.tile_pool(name="kp", bufs=2) as kp,
            tc.tile_pool(name="kzp", bufs=1) as kzp,
            tc.tile_pool(name="vp", bufs=2) as vp,
            tc.tile_pool(name="ep", bufs=2) as ep,
            tc.tile_pool(name="op", bufs=2) as opool,
            tc.tile_pool(name="rp", bufs=2) as rp,
            tc.tile_pool(name="fpl", bufs=3) as fpl,
            tc.tile_pool(name="psA", bufs=4, space="PSUM") as psA,
        ):
            # ---- PE warm-up: the HAM clock gate defaults to 1.2 GHz and only
            # un-throttles after ~3.4us of sustained matmul activity. Fill the
            # initial DMA wait with dummy matmuls so real work runs at 2.4 GHz.
            if WARMUP:
                wz = constp.tile([P, P], dt.bfloat16, name="warmz")
                nc.gpsimd.memset(wz[:], 0.0)
                ps_w = psA.tile([P, P], dt.float32, tag="ps", name="warm")
                for _ in range(48):
                    nc.tensor.matmul(ps_w[:], wz[:], wz[:], start=True, stop=True)

            # startup-critical DMAs first. wqQ / chunk-0 x live in per-ec
            # TILES (tile-granular dependency tracking) so chunk 0's first
            # ec-outer Q matmul unblocks after two small transfers instead
            # of the full 8.4MB of weights.
            wqQ = [constp.tile([P, E], dt.bfloat16, name=f"wqQ{ec}")
                   for ec in range(EC)]
            xt0 = [constp.tile([P, CT], dt.bfloat16, name=f"xt0_{ec}")
                   for ec in range(EC)]
            nc.sync.dma_start(xt0[0][:], xt_d[0][:, 0, :])
            nc.sync.dma_start(wqQ[0][:, 0:P], wqkv_d[:, 0, 0:P])
            nc.sync.dma_start(wqQ[0][:, P:E], wqkv_d[:, 0, P:E])
            for ec in range(1, EC):
                nc.sync.dma_start(wqQ[ec][:], wqkv_d[:, ec, 0:E])
                nc.sync.dma_start(xt0[ec][:], xt_d[0][:, ec, :])
            wq = constp.tile([P, EC, 2 * E], dt.bfloat16)  # K and V blocks
            for ec in range(EC):
                nc.sync.dma_start(wq[:, ec, 0:E], wqkv_d[:, ec, E:2 * E])
            for ec in range(EC):
                nc.sync.dma_start(wq[:, ec, E:2 * E], wqkv_d[:, ec, 2 * E:3 * E])
            oh = constp.tile([P, H, H], dt.bfloat16)
            nc.sync.dma_start(oh[:], oh_d[:])
            if PACK_DB and PACK_BCAST:
                sel2 = constp.tile([P, 4, P], dt.float16)
                nc.sync.dma_start(sel2[:], sel2_d[:])
            elif PACK_DB:
                sel2 = constp.tile([P, EC, P], dt.float16)
                nc.sync.dma_start(sel2[:], sel2_d[:])
            else:
                sel = constp.tile([H, EC, P], dt.float16)
                nc.sync.dma_start(sel[:], sel_d[:])
            wo = constp.tile([P, EC, E], dt.bfloat16)
            for ec in range(EC):
                nc.sync.dma_start(wo[:, ec, :], wout_d[:, ec, :])
            if has_bqk:
                bqk = constp.tile([P, 2, EC], dt.float32)
                nc.sync.dma_start(bqk[:], bqk_d[:])
            if has_bout:
                cb = constp.tile([P, 2, 512], dt.float32)
                nc.sync.dma_start(cb[:], cb_d[:])

            # kz zero halves never change: clear the two persistent tiles once.
            kz_tiles = []
            if not PACK_SCORES:
                for i in range(2):
                    kzt = kzp.tile([P, H, CT], dt.bfloat16, tag=f"kz{i}",
                                   name=f"kz{i}")
                    nc.gpsimd.memset(kzt[:], 0.0)
                    kz_tiles.append(kzt)

            def stage_a1(wi, k_tiles, kz_sb, q_sb, tq):
                """scores -> exp (quartered ACTs so the D chain can start early).

                PACK_SCORES: K=64 contractions, head h at base partition
                64*(h%2) of its eviction tile; consecutive head pairs run
                concurrently in the two PE row-groups, draining to psum
                slots (i, i+4) = different banks."""
                e_sb = ep.tile([P, H, W], dt.bfloat16, tag="e")
                for half in range(2):
                    ps_s = psA.tile([P, 8, W], dt.float32, tag="ps")
                    for pos in range(8):
                        h = half * 8 + pos
                        if PACK_SCORES:
                            fg, hf, rg = h // 4, (h % 4) // 2, h % 2
                            pr = slice(rg * D, rg * D + D)
                            nc.tensor.matmul(
                                ps_s[:, SLOT[pos], :tq],
                                k_tiles[fg][pr, hf, wi * W:(wi + 1) * W],
                                q_sb[pr, h // 2, wi * W:wi * W + tq],
                                start=True,
                                stop=True,
                            )
                        else:
                            nc.tensor.matmul(
                                ps_s[:, pos, :tq],
                                kz_sb[:, h, wi * W:(wi + 1) * W],
                                q_sb[:, h // 2, wi * W:wi * W + tq],
                                start=True,
                                stop=True,
                            )
                    for qq in range(2):
                        nc.scalar.activation(
                            e_sb[:, half * 8 + qq * 4:half * 8 + qq * 4 + 4, :tq],
                            ps_s[:, qq * 4:qq * 4 + 4, :tq], AF.Exp,
                        )
                return e_sb

            def stage_d16(e_sb, tq):
                """denominators by e-slot via one-hot matmuls, then recip ->
                f16 cast for the broadcast matmuls.

                PACK_DB: two 8-matmul accumulation groups col-tiled at PE
                columns 0/64 into separate PSUM banks, interleaved issue ->
                2-way concurrent. rd16 rows 64g+s hold 1/den of e-slot
                (g, s)."""
                if PACK_DB:
                    # M=16 one-hot stationaries (cols 8-15 all-zero) -- the
                    # proven stationary width; group g drains at PE columns
                    # 64g into PSUM bank g.
                    ps_d = psA.tile([P, 2, 512], dt.float32, tag="ps")
                    for s in range(8):
                        for g in range(2):
                            nc.tensor.matmul(
                                ps_d[64 * g:64 * g + 16, g, :tq],
                                oh[:, s, :],
                                e_sb[:, g * 8 + s, :tq],
                                start=(s == 0),
                                stop=(s == 7),
                            )
                    rd32 = rp.tile([P, W], dt.float32, tag="rd32")
                    rd16 = rp.tile([P, W], dt.float16, tag="rd16")
                    # bcast contracts 16 rows/group with zero weights on the
                    # top 8: pre-zero rd16 so those lanes can't inject NaN.
                    nc.gpsimd.memset(rd16[:], 0.0)
                    for g in range(2):
                        pr = slice(64 * g, 64 * g + 8)
                        nc.vector.reciprocal_approx_fast(rd32[pr, :tq],
                                                         ps_d[pr, g, :tq])
                        nc.vector.tensor_copy(rd16[pr, :tq], rd32[pr, :tq])
                    return rd16
                ps_d = psA.tile([H, W], dt.float32, tag="ps")
                for h in range(H):
                    nc.tensor.matmul(
                        ps_d[:, :tq], oh[:, h, :], e_sb[:, h, :tq],
                        start=(h == 0), stop=(h == H - 1),
                    )
                rd32 = rp.tile([H, W], dt.float32, tag="rd32")
                nc.vector.reciprocal_approx_fast(rd32[:, :tq], ps_d[:, :tq])
                rd16 = rp.tile([H, W], dt.float16, tag="rd16")
                nc.vector.tensor_copy(rd16[:, :tq], rd32[:, :tq])
                return rd16

            def stage_a2(wi, e_sb, v_sb, rd16, tq):
                """bcast recip -> AV (r evicted during AV) -> normalized O."""
                # broadcast recip to O shape: R_O[cc*128+p, tq] = rd[h(cc,p), tq]
                ps_r = psA.tile([P, EC, W], dt.float32, tag="ps")
                if PACK_DB and PACK_BCAST:
                    # K=16 selector matmuls, 2-way row-tiled at PE rows 0/64;
                    # pairs (cc, cc+4) stream concurrently, draining into
                    # different PSUM banks of ps_r.
                    for j in range(4):
                        for g in range(2):
                            cc = 4 * g + j
                            pr = slice(64 * g, 64 * g + 16)
                            nc.tensor.matmul(
                                ps_r[:, cc, :tq], sel2[pr, j, :], rd16[pr, :tq],
                                start=True, stop=True,
                            )
                elif PACK_DB:
                    for cc in range(EC):
                        nc.tensor.matmul(
                            ps_r[:, cc, :tq], sel2[:, cc, :], rd16[:, :tq],
                            start=True, stop=True,
                        )
                else:
                    for cc in range(EC):
                        nc.tensor.matmul(
                            ps_r[:, cc, :tq], sel[:, cc, :], rd16[:, :tq],
                            start=True, stop=True,
                        )
                r_sb = rp.tile([P, EC, W], dt.float16, tag="ro")
                nc.vector.tensor_copy(r_sb[:, :, :tq], ps_r[:, :, :tq])
                # AV halves write two SEPARATE psum tiles so o_h0's multiply
                # (DVE) can run while AV's second half still streams on PE --
                # the next window's out-projection then finds o_h0 ready.
                o_halves = []
                for hh in range(2):
                    ps_o = psA.tile([P, 4, W], dt.float32, tag="ps",
                                    name=f"ps_o{hh}")
                    for h in range(hh * 8, hh * 8 + 8):
                        cc = h // 2 - hh * 4
                        po = (h % 2) * D
                        nc.tensor.matmul(
                            ps_o[po:po + D, cc, :tq],
                            v_sb[:, wi, h // 8, (h % 8) * D:(h % 8) * D + D],
                            e_sb[:, hh * 8 + SLOT[h % 8], :tq],
                            start=True,
                            stop=True,
                        )
                    sl = slice(hh * 4, hh * 4 + 4)
                    o_h = opool.tile([P, 4, W], dt.bfloat16, tag=f"o{hh}",
                                     name=f"o{hh}")
                    nc.vector.tensor_mul(o_h[:, :, :tq], ps_o[:, :, :tq],
                                         r_sb[:, sl, :tq])
                    o_halves.append(o_h)
                return o_halves

            def bmm_one(ps_f, o_halves, fh, cc):
                nc.tensor.matmul(
                    ps_f[:, fh, :],
                    o_halves[cc // 4][:, cc % 4, :],
                    wo[:, cc, fh * 512:(fh + 1) * 512],
                    start=(cc == 0),
                    stop=(cc == EC - 1),
                )

            def stage_b_out_half(ps_f, row0, rows, fh):
                """evict + DMA one 512-feature half of the out projection."""
                f_sb = fpl.tile([P, 512], dt.float32, tag=f"f{fh}", name=f"f{fh}")
                if has_bout:
                    nc.vector.tensor_add(f_sb[:], ps_f[:, fh, :], cb[:, fh, :])
                else:
                    # on ScalarE: keeps DVE free for the recip/normalize chain
                    nc.scalar.activation(f_sb[:], ps_f[:, fh, :], AF.Copy)
                nc.sync.dma_start(
                    out_d[row0:row0 + rows, fh * 512:(fh + 1) * 512],
                    f_sb[:rows],
                )

            pend = None
            for c in range(n_chunks):
                tcv = min(s_out - c * CT, CT)  # valid tokens in this chunk
                if c == 0:
                    xt = None
                else:
                    xt = xp.tile([P, EC, CT], dt.bfloat16, tag="xt")
                    nc.sync.dma_start(xt[:], xt_d[c])

                def xt_ec(ec, sl=slice(None)):
                    return xt0[ec][:, sl] if c == 0 else xt[:, ec, sl]

                q_sb = qkp.tile([P, EC, CT], dt.bfloat16, tag="q")
                kz_sb = kz_tiles[c % 2] if not PACK_SCORES else None
                v_sb = vp.tile([P, CW, 2, 512], dt.bfloat16, tag="v")

                # ---- Q (feature-major): psum[f_tile, t] ----
                if c == 0:
                    # ec-outer so the first matmul only needs the first two
                    # small DMAs; uses all 4 psum slots as live accumulators.
                    ps_qs = [psA.tile([P, 2, 512], dt.float32, tag="ps",
                                      name=f"ps_q{i}")
                             for i in range(4)]
                    for ec in range(EC):
                        for fg in range(4):
                            for half in range(2):
                                ft = fg * 2 + half
                                nc.tensor.matmul(
                                    ps_qs[fg][:, half, :tcv],
                                    wqQ[ec][:, ft * P:ft * P + P],
                                    xt_ec(ec, slice(0, tcv)),
                                    start=(ec == 0),
                                    stop=(ec == EC - 1),
                                )
                    for fg in range(4):
                        ps = ps_qs[fg]
                        if has_bqk:
                            for half in range(2):
                                ft = fg * 2 + half
                                nc.scalar.activation(
                                    q_sb[:, ft, :tcv], ps[:, half, :tcv],
                                    AF.Identity, bias=bqk[:, 0, ft:ft + 1],
                                )
                        else:
                            nc.scalar.activation(
                                q_sb[:, fg * 2:fg * 2 + 2, :tcv],
                                ps[:, :, :tcv], AF.Copy,
                            )
                else:
                    for fg in range(4):
                        ps = psA.tile([P, 2, 512], dt.float32, tag="ps")
                        for half in range(2):
                            ft = fg * 2 + half
                            for ec in range(EC):
                                nc.tensor.matmul(
                                    ps[:, half, :tcv],
                                    wqQ[ec][:, ft * P:ft * P + P],
                                    xt_ec(ec, slice(0, tcv)),
                                    start=(ec == 0),
                                    stop=(ec == EC - 1),
                                )
                        if has_bqk:
                            for half in range(2):
                                ft = fg * 2 + half
                                nc.scalar.activation(
                                    q_sb[:, ft, :tcv], ps[:, half, :tcv],
                                    AF.Identity, bias=bqk[:, 0, ft:ft + 1],
                                )
                        else:
                            nc.scalar.activation(
                                q_sb[:, fg * 2:fg * 2 + 2, :tcv],
                                ps[:, :, :tcv], AF.Copy,
                            )

                # ---- K (feature-major). Evicted into k_tiles: head 2*ft+hh
                # lives at partitions 64hh.. of tile fg, half ft%2. Padded key
                # columns (last chunk) are memset to zero: the reference's
                # zero-padded x gives k=0 there, so padded keys contribute
                # exp(0)=1 with v=0. ----
                k_tiles = [kp.tile([P, 2, CT], dt.bfloat16, tag=f"k{fg}",
                                   name=f"k{fg}")
                           for fg in range(4)]
                for fg in range(4):
                    ps = psA.tile([P, 2, 512], dt.float32, tag="ps")
                    for half in range(2):
                        ft = fg * 2 + half
                        off = ft * P
                        for ec in range(EC):
                            nc.tensor.matmul(
                                ps[:, half, :tcv],
                                wq[:, ec, off:off + P],
                                xt_ec(ec, slice(0, tcv)),
                                start=(ec == 0),
                                stop=(ec == EC - 1),
                            )
                    if tcv < CT:
                        nc.gpsimd.memset(k_tiles[fg][:, :, tcv:], 0.0)
                    if has_bqk:
                        for half in range(2):
                            ft = fg * 2 + half
                            nc.scalar.activation(
                                k_tiles[fg][:, half, :tcv], ps[:, half, :tcv],
                                AF.Identity, bias=bqk[:, 1, ft:ft + 1],
                            )
                    else:
                        nc.scalar.activation(
                            k_tiles[fg][:, :, :tcv], ps[:, :, :tcv], AF.Copy,
                        )
                    if not PACK_SCORES:
                        for half in range(2):
                            ft = fg * 2 + half
                            for hh in range(2):
                                pr = slice(hh * 64, hh * 64 + 64)
                                nc.sync.dma_start(
                                    kz_sb[pr, 2 * ft + hh, :],
                                    k_tiles[fg][pr, half, :],
                                )

                # ---- V (token-major): psum[t, f] per window ----
                for wi in range(CW):
                    ps = psA.tile([P, 2, 512], dt.float32, tag="ps")
                    for fh in range(2):
                        off = E + fh * 512
                        for ec in range(EC):
                            nc.tensor.matmul(
                                ps[:, fh, :],
                                xt_ec(ec, slice(wi * W, (wi + 1) * W)),
                                wq[:, ec, off:off + 512],
                                start=(ec == 0),
                                stop=(ec == EC - 1),
                            )
                    nc.vector.tensor_copy(v_sb[:, wi], ps[:])

                # ---- attention (A) + out-projection (B), software-pipelined:
                # B(w-1) is emitted inside A(w) so the PE has big streams to
                # hide the D-chain ldweights and the evict->normalize latency.
                for wi in range(CW):
                    g = c * CW + wi
                    row0 = g * W
                    rows = min(s_out - row0, W)
                    if rows <= 0:
                        continue
                    tq = rows
                    e_sb = stage_a1(wi, k_tiles, kz_sb, q_sb, tq)
                    if pend is not None:
                        ps_f = psA.tile([P, 2, 512], dt.float32, tag="ps")
                        for cc in range(EC):
                            bmm_one(ps_f, pend[0], 0, cc)
                    rd16 = stage_d16(e_sb, tq)
                    if pend is not None:
                        stage_b_out_half(ps_f, pend[1], pend[2], 0)
                        for cc in range(EC):
                            bmm_one(ps_f, pend[0], 1, cc)
                        stage_b_out_half(ps_f, pend[1], pend[2], 1)
                    o_halves = stage_a2(wi, e_sb, v_sb, rd16, tq)
                    pend = (o_halves, row0, rows)

            if pend is not None:
                ps_f = psA.tile([P, 2, 512], dt.float32, tag="ps")
                for cc in range(EC):
                    bmm_one(ps_f, pend[0], 0, cc)
                stage_b_out_half(ps_f, pend[1], pend[2], 0)
                for cc in range(EC):
                    bmm_one(ps_f, pend[0], 1, cc)
                stage_b_out_half(ps_f, pend[1], pend[2], 1)

    nc.compile()
    return nc


def prep_inputs(x, w_qkv, b_qkv, w_out, b_out, n_chunks, s_out):
    """Host-side staging: pad, transpose, cast, fold scale into w_q."""
    sp = n_chunks * CT
    nb = x.shape[0]

    wqkvT = np.ascontiguousarray(w_qkv.T).astype(np.float32).copy()
    wqkvT[:, :E] *= 1.0 / np.sqrt(D)
    wqkv_sb = np.ascontiguousarray(
        wqkvT.reshape(EC, P, 3 * E).transpose(1, 0, 2)
    ).astype(BF16)

    woutT = np.ascontiguousarray(w_out.T)
    wout_sb = np.ascontiguousarray(
        woutT.reshape(EC, P, E).transpose(1, 0, 2)
    ).astype(BF16)

    oh = np.zeros((P, H, H), dtype=BF16)
    for h in range(H):
        oh[:, h, h] = 1

    base = {"wqkv": wqkv_sb, "wout": wout_sb, "onehot": oh}

    if PACK_DB and PACK_BCAST:
        # rd16 row 64g+s holds 1/den of e-slot (g, s) = head 8g + POS[s];
        # e-chunk cc = 4g+j output partition p wants head 2cc + p//64
        # -> s = j (p<64) or j+4 (p>=64).
        sel2 = np.zeros((P, 4, P), dtype=F16)
        for g in range(2):
            for j in range(4):
                sel2[64 * g + j, j, 0:64] = 1
                sel2[64 * g + j + 4, j, 64:128] = 1
        base["sel2"] = sel2
    elif PACK_DB:
        sel2 = np.zeros((P, EC, P), dtype=F16)
        for cc in range(EC):
            g, j = cc // 4, cc % 4
            sel2[64 * g + j, cc, 0:64] = 1
            sel2[64 * g + j + 4, cc, 64:128] = 1
        base["sel2"] = sel2
    else:
        # rd16 row r holds 1/den of e-slot r = head (r//8)*8 + POS[r%8];
        # output partition p of chunk cc wants head 2cc + p//64.
        selm = np.zeros((H, EC, P), dtype=F16)
        for cc in range(EC):
            for m in range(P):
                h = 2 * cc + m // D
                r = (h // 8) * 8 + SLOT[h % 8]
                selm[r, cc, m] = 1
        base["sel"] = selm

    has_bqk = bool(np.any(b_qkv[:2 * E]))
    has_bout = bool(np.any(b_out)) or bool(np.any(b_qkv[2 * E:]))
    if has_bqk:
        bqk = np.stack(
            [b_qkv[:E].reshape(EC, P).T / np.sqrt(D),
             b_qkv[E:2 * E].reshape(EC, P).T], axis=1
        ).astype(np.float32)  # (P, 2, EC)
        base["bqk"] = np.ascontiguousarray(bqk)
    if has_bout:
        cbv = (b_out + b_qkv[2 * E:] @ w_out.T).astype(np.float32)  # (E,)
        base["cb"] = np.ascontiguousarray(
            np.broadcast_to(cbv.reshape(1, 2, 512), (P, 2, 512))
        ).copy()

    in_maps = []
    for b in range(nb):
        xp_ = np.zeros((sp, E), dtype=np.float32)
        xp_[:min(s_out, x.shape[1])] = x[b][:s_out]
        xT = np.ascontiguousarray(xp_.T)  # (E, sp)
        xt_sb = np.ascontiguousarray(
            xT.reshape(EC, P, n_chunks, CT).transpose(2, 1, 0, 3)
        ).astype(BF16)  # (n_chunks, P, EC, CT)
        m = dict(base)
        m["xt"] = xt_sb
        in_maps.append(m)
    return in_maps, has_bqk, has_bout


def run(x, w_qkv, b_qkv, w_out, b_out, n_chunks=NW // CW, s_out=S, trace=False):
    from concourse import bass_utils

    in_maps, has_bqk, has_bout = prep_inputs(
        x, w_qkv, b_qkv, w_out, b_out, n_chunks, s_out
    )
    key = (n_chunks, s_out, has_bqk, has_bout)
    if key not in _cache:
        _cache[key] = build_nc(*key)
    nc = _cache[key]

    res = bass_utils.run_bass_kernel_spmd(
        nc, in_maps, core_ids=list(range(len(in_maps))), trace=trace,
    )
    out = np.stack([r["out"] for r in res.results], axis=0)
    return out, res


def kernel(x, w_qkv, b_qkv, w_out, b_out):
    x = np.asarray(x, dtype=np.float32)
    w_qkv = np.asarray(w_qkv, dtype=np.float32)
    b_qkv = np.asarray(b_qkv, dtype=np.float32)
    w_out = np.asarray(w_out, dtype=np.float32)
    b_out = np.asarray(b_out, dtype=np.float32)
    out, _ = run(x, w_qkv, b_qkv, w_out, b_out)
    return out
